# revision 12
# baseline (speedup 1.0000x reference)
"""Trainium2 Bass kernel for nn_Attention_16612933501279.

Algebraic refactor (exact in fp32; bf16 device compute):
    s'[b,n,p]  = rsqrt(ssq)            (ssq = sum_c c^2; s = sqrt(C)*s')
    wq2[b]     = (q @ Wq.T) @ (Wkv[:D] * g) * sqrt(C) / sqrt(D)   (host)
    dots       = (wq2[b] . c[b,n,:,p]) * s'
    att        = softmax_n(dots);  w = att * s'
    Mw2        = Wo @ (Wkv[D:] * g) * sqrt(C)                     (host)
    out[:,p]   = Mw2 @ (sum_n w[n,p] * c[b,n,:,p]) + bo

Device mapping (per core, H-sharded 8 ways):
  - c stored kc-major ([128, B, KC, N, PIX]) so squares and w-products
    are fully contiguous [128, 2048] DVE ops (strided/broadcast APs run
    ~4x slower on DVE).
  - input DMA'd in 16 quarter-batch chunks, issued first, chained at
    depth 6 (completion-serialized chains add ~2us/link bubbles).
  - ~36 warmup matmuls on consts keep the PE HAM clock at 2.4 GHz
    before real work arrives.
  - stats: one-hot-column matmuls, ssq at col-strip 0 / dots at strip
    32 run concurrently via tile_position. b0+b1 share a stacked
    [16,PIX] softmax chain; b2, b3 get solo [8,PIX] chains.
  - rsqrt: quadratic seed + one Newton step (custom DVE ops) on raw
    ssq (eps dropped; sqrt(C) folded into host consts).
  - w broadcast to 128 partitions via DMA from a DRAM bounce of w
    (SBUF APs can't have 0-stride partition dims; DRAM APs can).
  - context mix: prod = c*w_bcast [DVE bf16 2x], accumulated over n in
    PSUM via identity matmul; final projection + bias [ACT]; bf16 out.
"""

import sys

import numpy as np

try:
    import concourse.bass as bass  # noqa: F401
except ImportError:  # harness runs from a fresh dir; concourse lives here
    sys.path.insert(0, "/opt/trn_rl_repo")

import concourse.bass as bass
import concourse.mybir as mybir
from concourse import bacc, tile
from concourse import dve_ops as _dve_ops
from concourse.bass_utils import run_bass_kernel_spmd
from concourse.dve_ops import DveOp
from concourse.dve_spec import C0, C1, C2, Spec, Src0, Src1, lower, sq
from concourse.dve_spec import _has_src1 as has_src1
from concourse.dve_uop import DveOpSpec

AF = mybir.ActivationFunctionType
ALU = mybir.AluOpType
BF16 = mybir.dt.bfloat16
F32 = mybir.dt.float32

B, N, C, H, W = 4, 8, 256, 64, 64
D = 512
NCORES = 8
HS = H // NCORES          # 8 rows of H per core
PIX = HS * W              # 512 pixels per (b, n) tile per core
KC = C // 128             # 2 contraction chunks of 128 channels
NH = N // 2               # tokens per half (squares/prods granularity)

N_WARM = 36               # PE warmup matmuls

# rsqrt quad seed for rsqrt(t) on t in [0.5, 1.8], applied to raw ssq
QC0, QC1, QC2 = 1.91393121, -1.22982285, 0.33246410
SQRT_C = float(np.sqrt(C))
QA = QC2 / (C * C * SQRT_C)
QB = QC1 / (C * SQRT_C)
QD = QC0 / SQRT_C

# const tile free-axis layout (bf16 elements)
DOTS_OFF = 0                          # (pair, kc, db, n) -> [128,16]
N_DP = 2 * KC * 2 * N
SSQA_OFF = DOTS_OFF + N_DP * 16       # (db, n) -> [128,16], shared pairs
ID_OFF = SSQA_OFF + 16 * 16
ZONESA_OFF = ID_OFF + 128             # [16,16] blockdiag ones
MWT_OFF = ZONESA_OFF + 16
CONST_W = MWT_OFF + KC * C

# squares engine per (b, kc, half): v=DVE, a=ACT, g=GPSIMD
SQ_ENG = {
    (0, 0, 0): "v", (0, 1, 0): "a", (0, 0, 1): "a", (0, 1, 1): "a",
    (1, 0, 0): "v", (1, 1, 0): "a", (1, 0, 1): "a", (1, 1, 1): "v",
    (2, 0, 0): "v", (2, 1, 0): "a", (2, 0, 1): "v", (2, 1, 1): "a",
    (3, 0, 0): "v", (3, 1, 0): "a", (3, 0, 1): "v", (3, 1, 1): "v",
}


def _register_op(name, spec_body, spec_ref):
    for op in _dve_ops.OPS:
        if op.name == name:
            return op
    spec = Spec(body=spec_body, reference=spec_ref)
    sub = _dve_ops._CUSTOM_DVE_ROW_BASE + len(_dve_ops.OPS)
    assert sub < 0x20
    shas = {}
    for ver in ("v3", "v4"):
        try:
            s = DveOpSpec(name=name, opcode=sub, uops=lower(spec, ver=ver),
                          rd1_en=has_src1(spec))
            shas[ver] = s.sha(ver)
        except Exception:
            pass
    op = DveOp(name, spec, subdim=False, uops_sha=shas)
    _dve_ops.OPS.append(op)
    _dve_ops._SUB_OPCODE_FOR_NAME[name] = sub
    _dve_ops.CUSTOM_DVE_SPECS[name] = spec
    return op


RSQRT_NR = _register_op(
    "ANT_RSQRT_NR_ATT",
    Src1 * ((Src0 * C0 + C1) * sq(Src1) + C2),
    lambda in0, in1, c0, c1, c2: in1 * ((in0 * c0 + c1) * in1 * in1 + c2),
)

RSQRT_QSEED = _register_op(
    "ANT_RSQRT_QSEED_ATT",
    (Src0 * C0 + C1) * Src0 + C2,
    lambda in0, in1, c0, c1, c2: (in0 * c0 + c1) * in0 + c2,
)


def _build_nc():
    nc = bacc.Bacc(None, target_bir_lowering=False)
    c_d = nc.declare_dram_parameter("c", [128, B, KC, N, PIX], BF16, isOutput=False)
    k_d = nc.declare_dram_parameter("consts", [128, CONST_W], BF16, isOutput=False)
    bo_d = nc.declare_dram_parameter("bo2", [128, KC], F32, isOutput=False)
    out_d = nc.declare_dram_parameter("out", [B, C, HS, W], BF16, isOutput=True)
    w_dram = nc.dram_tensor("w_scratch", [B, N, PIX], BF16, kind="Internal")

    with (
        tile.TileContext(nc) as tc,
        tc.tile_pool(name="const", bufs=1) as cpool,
        tc.tile_pool(name="work", bufs=4) as work,
        tc.tile_pool(name="small", bufs=3) as small,
        tc.tile_pool(name="psum", bufs=1, space="PSUM") as pp,
    ):
        # ---- input c: 16 quarter-batch chunks, depth-6 chained; the
        # consts DMA is interleaved after the first two chunks ----
        consts = cpool.tile([128, CONST_W], BF16, tag="consts")
        bo_sb = cpool.tile([128, KC], F32, tag="bo")
        c_sb = [cpool.tile([128, KC, N, PIX], BF16, tag=f"c{b}", name=f"c{b}")
                for b in range(B)]
        cdmas = []
        for b in range(B):
            for qd in range(4):
                n0 = 2 * qd
                ins = nc.sync.dma_start(
                    c_sb[b][:, :, n0 : n0 + 2], c_d[:, b, :, n0 : n0 + 2]
                )
                if len(cdmas) >= 3:
                    tile.add_dep_helper(
                        ins.ins, cdmas[-3].ins,
                        reason="pipeline input DMAs depth-3",
                    )
                cdmas.append(ins)
                if b == 0 and qd == 1:
                    nc.sync.dma_start(consts[:], k_d[:])
                    nc.sync.dma_start(bo_sb[:], bo_d[:])

        def st_dots_a(pair, kc, db, n):
            o = DOTS_OFF + (((pair * KC + kc) * 2 + db) * N + n) * 16
            return consts[:, o : o + 16]

        def st_ssq_a(db, n):
            o = SSQA_OFF + (db * N + n) * 16
            return consts[:, o : o + 16]

        ident = consts[:, ID_OFF : ID_OFF + 128]
        zones_a = consts[0:16, ZONESA_OFF : ZONESA_OFF + 16]

        def st_mwt(kc, mc):
            o = MWT_OFF + kc * C + mc * 128
            return consts[:, o : o + 128]

        # PSUM: 2 stats + 2 z + 4 mix = 8 banks
        stats = {}
        stats["A"] = pp.tile([48, PIX], F32, tag="stats", bufs=2, name="statsA")
        stats["B"] = pp.tile([48, PIX], F32, tag="stats", bufs=2, name="statsB")

        # ---- PE warmup: keep HAM at full clock until real work ----
        zwarm = pp.tile([16, PIX], F32, tag="z", bufs=2, name="zwarm")
        for _ in range(N_WARM):
            nc.tensor.matmul(zwarm[:], st_ssq_a(0, 0), consts[:, 0:PIX],
                             start=True, stop=True)

        sq_done = {}

        def emit_squares(b, kc, h):
            """csq for tokens [4h, 4h+4) of chunk kc -- contiguous."""
            n0 = NH * h
            csq = work.tile([128, NH, PIX], BF16, tag="csq", bufs=4,
                            name="csq")
            src_ = c_sb[b][:, kc, n0 : n0 + NH]
            eng = {"v": nc.vector, "g": nc.gpsimd, "a": None}[SQ_ENG[(b, kc, h)]]
            if eng is None:
                nc.scalar.activation(csq[:], src_, AF.Square)
            else:
                eng.tensor_mul(csq[:], src_, src_)
            sq_done[(b, kc, h)] = csq

        def emit_stats(b, kc, h):
            csq = sq_done[(b, kc, h)]
            n0 = NH * h
            pair, db = divmod(b, 2)
            stp = stats["A" if pair == 0 else "B"]
            ssq_out, dots_out = stp[0:16, :], stp[32:48, :]
            for j in range(NH):
                n = n0 + j
                fr = db == 0 and n == 0 and kc == 0
                la = db == 1 and n == N - 1 and kc == KC - 1
                nc.tensor.matmul(
                    ssq_out, st_ssq_a(db, n), csq[:, j, :],
                    start=fr, stop=la, tile_position=(0, 0),
                )
                nc.tensor.matmul(
                    dots_out, st_dots_a(pair, kc, db, n),
                    c_sb[b][:, kc, n, :],
                    start=fr, stop=la, tile_position=(0, 32),
                )

        w_store = {}

        def emit_chain(group):
            stp = stats[group]
            rows = 16
            zone = zones_a
            ssq = stp[0:rows, :]
            dots = stp[32 : 32 + rows, :]
            y0 = small.tile([rows, PIX], F32, tag="y0")
            nc.vector._custom_dve(
                RSQRT_QSEED, out=y0[:], in0=ssq, in1=None,
                s0=QA, s1=QB, imm2=QD,
            )
            s_sb = small.tile([rows, PIX], BF16, tag="s")
            nc.vector._custom_dve(
                RSQRT_NR, out=s_sb[:], in0=ssq, in1=y0[:],
                s0=-0.5, s1=0.0, imm2=1.5,
            )
            dscl = small.tile([rows, PIX], F32, tag="dscl")
            nc.vector.tensor_mul(dscl[:], dots, s_sb[:])
            e_sb = small.tile([rows, PIX], BF16, tag="e", bufs=2)
            nc.scalar.activation(e_sb[:], dscl[:], AF.Exp)
            zpt = pp.tile([rows, PIX], F32, tag="z", bufs=2, name="z")
            nc.tensor.matmul(zpt[:], zone, e_sb[:], start=True, stop=True)
            es = small.tile([rows, PIX], BF16, tag="es")
            nc.vector.tensor_mul(es[:], e_sb[:], s_sb[:])
            zinv = small.tile([rows, PIX], F32, tag="zinv")
            nc.vector.reciprocal_approx_fast(zinv[:], zpt[:])
            w_sb = small.tile([rows, PIX], BF16, tag="w", bufs=2)
            nc.vector.tensor_mul(w_sb[:], es[:], zinv[:])
            b0_, b1_ = (0, 1) if group == "A" else (2, 3)
            nc.sync.dma_start(w_dram[b0_], w_sb[0:8, :])
            nc.sync.dma_start(w_dram[b1_], w_sb[8:16, :])

        prod_store = {}

        def emit_bcast(b):
            wbt = work.tile([128, N, PIX], BF16, tag="wbt", bufs=2,
                            name="wbt")
            for h in range(2):
                n0 = NH * h
                nc.sync.dma_start(
                    wbt[:, n0 : n0 + NH, :],
                    w_dram[b][None, n0 : n0 + NH, :].to_broadcast(
                        (128, NH, PIX)
                    ),
                )
            return wbt

        def emit_prods(b, wbt):
            prod = work.tile([128, KC, N, PIX], BF16, tag="prod", bufs=2,
                             name="prod")
            for h in range(2):
                n0 = NH * h
                for kc in range(KC):
                    nc.vector.tensor_mul(
                        prod[:, kc, n0 : n0 + NH],
                        c_sb[b][:, kc, n0 : n0 + NH],
                        wbt[:, n0 : n0 + NH, :],
                    )
            prod_store[b] = prod

        def emit_accum_out(b):
            prod = prod_store[b]
            cm = [pp.tile([128, PIX], F32, tag="mix", name=f"cm{kc}",
                          bufs=4) for kc in range(KC)]
            for n in range(N):
                for kc in range(KC):
                    nc.tensor.matmul(
                        cm[kc][:], ident, prod[:, kc, n, :],
                        start=(n == 0), stop=(n == N - 1),
                    )
            cmix = work.tile([128, KC, PIX], BF16, tag="cmix", bufs=2,
                             name="cmix")
            nc.scalar.copy(cmix[:, 0, :], cm[0][:])
            nc.scalar.copy(cmix[:, 1, :], cm[1][:])
            osb = work.tile([128, KC, PIX], BF16, tag="osb", bufs=2,
                            name="osb")
            for mc in range(KC):
                ops = pp.tile([128, PIX], F32, tag="mix", name="ops", bufs=4)
                for kc in range(KC):
                    nc.tensor.matmul(
                        ops[:], st_mwt(kc, mc), cmix[:, kc, :],
                        start=(kc == 0), stop=(kc == KC - 1),
                    )
                nc.scalar.activation(
                    osb[:, mc, :], ops[:], AF.Identity,
                    bias=bo_sb[:, mc : mc + 1], scale=1.0,
                )
                nc.scalar.dma_start(
                    out_d[b].rearrange("(mc p) h w -> mc p (h w)", p=128)[mc],
                    osb[:, mc, :],
                )

        def emit_batch_stats(b):
            # square/stats in input-chunk arrival order: (kc0,h), (kc1,h)
            for h in range(2):
                for kc in range(KC):
                    emit_squares(b, kc, h)
                    emit_stats(b, kc, h)

        # ---- emission order (priority hints for the Tile scheduler):
        # squares/stats for every batch first, then the softmax chains
        # (ready earliest, gate the slow DMA broadcast pipelines), then
        # bcasts+prods, then PSUM accumulation/projection ----
        emit_batch_stats(0)
        emit_batch_stats(1)
        emit_batch_stats(2)
        emit_chain("A")
        emit_batch_stats(3)
        emit_chain("B")
        wbt0 = emit_bcast(0)
        emit_prods(0, wbt0)
        wbt1 = emit_bcast(1)
        emit_prods(1, wbt1)
        wbt2 = emit_bcast(2)
        emit_prods(2, wbt2)
        emit_accum_out(0)
        wbt3 = emit_bcast(3)
        emit_prods(3, wbt3)
        emit_accum_out(1)
        emit_accum_out(2)
        emit_accum_out(3)

    nc.compile()
    return nc


def _host_consts(q, g, Wq, Wkv, Wo, bo):
    bf = mybir.dt.np(BF16)
    q, g, Wq, Wkv, Wo, bo = (
        np.asarray(x, np.float32) for x in (q, g, Wq, Wkv, Wo, bo)
    )
    Wk_g = Wkv[:D] * g[None, :]
    Wv_g = Wkv[D:] * g[None, :]
    wq2 = ((q @ Wq.T) @ Wk_g) * (D ** -0.5) * SQRT_C   # [B, C]
    Mw2 = (Wo @ Wv_g) * SQRT_C                         # [C, C]

    consts = np.zeros((128, CONST_W), np.float32)
    for pair in range(2):
        for kc in range(KC):
            for db in range(2):
                b = 2 * pair + db
                for n in range(N):
                    o = DOTS_OFF + (((pair * KC + kc) * 2 + db) * N + n) * 16
                    consts[:, o + 8 * db + n] = wq2[b, kc * 128 : (kc + 1) * 128]
    for db in range(2):
        for n in range(N):
            o = SSQA_OFF + (db * N + n) * 16
            consts[:, o + 8 * db + n] = 1.0
    consts[:, ID_OFF : ID_OFF + 128] = np.eye(128, dtype=np.float32)
    for db in range(2):
        consts[8 * db : 8 * db + 8,
               ZONESA_OFF + 8 * db : ZONESA_OFF + 8 * db + 8] = 1.0
    for kc in range(KC):
        consts[:, MWT_OFF + kc * C : MWT_OFF + (kc + 1) * C] = Mw2[
            :, kc * 128 : (kc + 1) * 128
        ].T
    bo2 = np.zeros((128, KC), np.float32)
    bo2[:, :KC] = bo.reshape(KC, 128).T
    return consts.astype(bf), bo2


_NC_CACHE = {}


def _get_nc():
    if "nc" not in _NC_CACHE:
        _NC_CACHE["nc"] = _build_nc()
    return _NC_CACHE["nc"]


def _run(q, c, g, Wq, Wkv, Wo, bo, trace=False):
    bf = mybir.dt.np(BF16)
    consts, bo2 = _host_consts(q, g, Wq, Wkv, Wo, bo)
    c_bf = np.asarray(c, np.float32).astype(bf)
    # [B,N,C,H,W] -> per core [128, B, KC, N, PIX] (kc-major SBUF layout)
    c_t = c_bf.reshape(B, N, KC, 128, H // HS, HS * W).transpose(4, 3, 0, 2, 1, 5)
    in_maps = []
    for i in range(NCORES):
        shard = np.ascontiguousarray(c_t[i])
        in_maps.append({"c": shard, "consts": consts, "bo2": bo2})
    nc = _get_nc()
    res = run_bass_kernel_spmd(nc, in_maps, core_ids=list(range(NCORES)),
                               trace=trace)
    out = np.concatenate(
        [np.asarray(res.results[i]["out"]).astype(np.float32)
         for i in range(NCORES)],
        axis=2,
    )
    return out, res


def kernel(q, c, g, Wq, Wkv, Wo, bo):
    out, _ = _run(q, c, g, Wq, Wkv, Wo, bo, trace=False)
    return out


def kernel_traced(q, c, g, Wq, Wkv, Wo, bo):
    out, res = _run(q, c, g, Wq, Wkv, Wo, bo, trace=True)
    return out, res
